# revision 13
# baseline (speedup 1.0000x reference)
"""Llama SDPA attention (GQA + RoPE + causal) on 8 Trainium2 NeuronCores.

Sharding: DP-2 over batch x TP-4 over heads. Core c = 4*b + g handles
batch b and head group g (8 q heads, 2 kv heads). Each core computes its
partial o_proj output [T, C] (Wo split along the input-feature dim); the
partials are summed on the host (the gather/unshard step).

Device kernel (all matmul operands bf16; fp32 PSUM accumulation):
  - Phase A: Q/K projections in 512-token chunks; weights stationary,
    x^T moving (tokens on the free dim). RoPE via one ACT PSUM->SBUF
    bf16 evacuation + 4 partition-shifted bf16 SBUF copies + bf16
    mul/mul/add on DVE (2-4x modes); q packed into per-kv head slots.
  - v^T computed DIRECTLY in [keys, features] layout: x-chunk stationary
    [C,128 tok], wv^T moving [C,128 feat] -> PSUM [tok, feat]; two DVE
    copies split kv0/kv1 into v1 slots (no PE transposes, no ident).
    v1's appended ones-block (64 cols) makes the PV matmul emit softmax
    denominators broadcast across 64 partitions for free.
  - Phase B per 128-query strip: QK as zero-padded full-128-contraction
    matmuls (qk_mode='padded', the default: k rows for the other kv
    group are zeros, so QK/PV/o_proj all stay in the 128x128 PE config
    and never pay a tiling-mode drain; qk_mode='tiled' uses row-tiled
    64-contraction pairs instead, ~8% slower on HW). One [128,1024] exp
    on ScalarE per key tile covers both kv groups; PV accumulates in
    PSUM over key tiles; normalization = reciprocal + mul per kv.
  - o_proj is deferred by one strip so its matmuls/evacuations overlap
    the next strip's attention; output staged bf16, partials summed on
    host in fp32.
  - DMAs split across the two HWDGE queues: x loads + part stores on
    SP, weights/tables on ACT; first weight ci-group lands before x's
    second half so the first matmul group starts early.
  - Phases are emitted sequentially: a fine-grained A/B interleave (the
    bq/drain machinery below, now pinned to per=0) measured 1.6x SLOWER
    on HW despite a better simulated schedule - cross-phase dependency
    convoys and extra PE mode switches dominate on silicon.
"""

import sys
import numpy as np

if '/opt/trn_rl_repo' not in sys.path:
    sys.path.insert(0, '/opt/trn_rl_repo')

B, T, C = 2, 2048, 2048
NH, NKV, HD = 32, 8, 64
G = 4              # head groups (TP degree)
QF = NH // G * HD  # 512 q features per core
KF = NKV // G * HD # 128 k/v features per core
NE = 4             # token chunks in projection phase
TE = T // NE       # 512
NCC = C // 128     # 16 contraction chunks
NG = 4             # weight ci-groups (startup overlap)
CG = NCC // NG
NTS = T // 128     # 16 token strips (attention)
SLOT = {0: 0, 1: 2, 2: 1, 3: 3}

_CACHE = {}


def _rope_perm(nheads):
    """Per-head feature permutation: [d0,d2,...,d62, d1,d3,...,d63]."""
    p = []
    for h in range(nheads):
        base = h * HD
        p.extend(base + d for d in range(0, HD, 2))
        p.extend(base + d for d in range(1, HD, 2))
    return np.array(p, dtype=np.int64)


def _wo_perm():
    """att feature r_new = s*128 + 64*kv + d  <-  old h_l*64 + d."""
    idx = np.empty(QF, dtype=np.int64)
    for s in range(4):
        for kv in range(2):
            for d in range(HD):
                h_l = 4 * kv + SLOT[s]
                idx[s * 128 + 64 * kv + d] = h_l * HD + d
    return idx


def _build_program(causal, rep=1, qk_mode="padded"):
    import concourse.bass as bass
    import concourse.tile as tile
    import concourse.mybir as mybir
    from concourse import bacc
    from concourse.bass import ts

    F32 = mybir.dt.float32
    BF16 = mybir.dt.bfloat16
    Exp = mybir.ActivationFunctionType.Exp
    Copy = mybir.ActivationFunctionType.Copy
    padded = qk_mode == "padded"

    nc = bacc.Bacc("TRN2", target_bir_lowering=False, debug=False)

    # pre-tiled inputs: per-partition data is contiguous in DRAM
    xTd = nc.dram_tensor("xTd", [128, NE, NCC, TE], BF16, kind="ExternalInput").ap()
    wqd = nc.dram_tensor("wqd", [NG, 128, CG, QF], BF16, kind="ExternalInput").ap()
    wkd = nc.dram_tensor("wkd", [NG, 128, CG, KF], BF16, kind="ExternalInput").ap()
    wvd = nc.dram_tensor("wvd", [NG, 128, CG, KF], BF16, kind="ExternalInput").ap()
    wod = nc.dram_tensor("wod", [128, QF // 128, C], BF16, kind="ExternalInput").ap()
    c2 = nc.dram_tensor("c2", [128, T], BF16, kind="ExternalInput").ap()
    s2 = nc.dram_tensor("s2", [128, T], BF16, kind="ExternalInput").ap()
    maskd = nc.dram_tensor("maskd", [128, 8, 128], F32, kind="ExternalInput").ap()
    if not causal:
        maskT = nc.dram_tensor("maskT", [T, T], F32, kind="ExternalInput").ap()
    part = nc.dram_tensor("part", [T, C], BF16, kind="ExternalOutput").ap()

    with tile.TileContext(nc) as tc:
        from contextlib import ExitStack
        with ExitStack() as ctx:
            persist = ctx.enter_context(tc.tile_pool(name="persist", bufs=1))
            # roped k^T; padded: [p, kv, t] with off-group rows zeroed
            if padded:
                kT_sbp = persist.tile([128, 2, T], BF16)
                nc.vector.memset(kT_sbp[64:128, 0, :], 0.0)
                nc.vector.memset(kT_sbp[0:64, 1, :], 0.0)
            else:
                kT_sbp = persist.tile([128, T], BF16)
            qp_sb = persist.tile([128, 4, T], BF16)  # packed q^T [64*kv+d, slot, t]
            # [keys, kv, j, 0:64]=v^T ; [..., 64:128]=1.0 (PV emits sums
            # broadcast into partitions 64..128)
            v1_sb = persist.tile([128, 2, NTS, 128], BF16)
            nc.vector.memset(v1_sb[:, :, :, HD:128], 1.0)
            maskd_sb = persist.tile([128, 8, 128], F32)
            nc.scalar.dma_start(maskd_sb, maskd)
            wo_sb = persist.tile([128, QF // 128, C], BF16)

            for _rep in range(rep):
                # Phases A (proj+RoPE+v) and B (attention+o_proj) are
                # emitted INTERLEAVED: strip c's attention is eligible
                # once chunk c//4 closes, so phase A's PE-heavy stream
                # overlaps phase B's ACT-heavy exp stream. One shared
                # PSUM accumulation ring (psA: projections, v, o_proj)
                # + psS (QK) + psO (PV) = 2+2+4+2 = 8 banks... psA 2.
                with tc.tile_pool(name="stage_a", bufs=1) as stage_a, \
                     tc.tile_pool(name="weights", bufs=1) as wpool, \
                     tc.tile_pool(name="xpool", bufs=2) as xpool, \
                     tc.tile_pool(name="rtmp", bufs=3) as rpool, \
                     tc.tile_pool(name="pp", bufs=3) as ppool, \
                     tc.tile_pool(name="norm", bufs=2) as npool, \
                     tc.tile_pool(name="atts", bufs=3) as apool, \
                     tc.tile_pool(name="mload", bufs=2) as mpool, \
                     tc.tile_pool(name="outs", bufs=2) as opool, \
                     tc.tile_pool(name="psA", bufs=2, space="PSUM") as psA, \
                     tc.tile_pool(name="psS", bufs=2, space="PSUM") as psS, \
                     tc.tile_pool(name="psO", bufs=2, space="PSUM") as psO:
                    c2_sb = stage_a.tile([128, T], BF16)
                    s2_sb = stage_a.tile([128, T], BF16)

                    # DMA order: x first half, first weight group (so the
                    # first matmul group waits only for its own operands),
                    # then the rest. x + part on SP queue; weights/tables
                    # on the ACT queue.
                    x_a = xpool.tile([128, 8, TE], BF16, tag="xa", name="x_a")
                    nc.sync.dma_start(x_a, xTd[:, 0, 0:8, :])
                    wq_g, wk_g, wv_g = [], [], []
                    x_b = None
                    for g in range(NG):
                        wqg = wpool.tile([128, CG, QF], BF16,
                                         name=f"wq{g}", tag=f"wq{g}")
                        nc.scalar.dma_start(wqg, wqd[g])
                        wq_g.append(wqg)
                        wkg = wpool.tile([128, CG, KF], BF16,
                                         name=f"wk{g}", tag=f"wk{g}")
                        nc.scalar.dma_start(wkg, wkd[g])
                        wk_g.append(wkg)
                        wvg = wpool.tile([128, CG, KF], BF16,
                                         name=f"wv{g}", tag=f"wv{g}")
                        nc.scalar.dma_start(wvg, wvd[g])
                        wv_g.append(wvg)
                        if g == 0:
                            x_b = xpool.tile([128, 8, TE], BF16, tag="xb",
                                             name="x_b")
                            nc.sync.dma_start(x_b, xTd[:, 0, 8:16, :])
                            nc.scalar.dma_start(c2_sb, c2)
                            nc.scalar.dma_start(s2_sb, s2)
                    x0 = (x_a, x_b)
                    nc.scalar.dma_start(wo_sb, wod)

                    def emit_proj_group(e, f, x_a, x_b):
                        ps = psA.tile([128, TE], F32, tag="acc")
                        for ci in range(NCC):
                            g, cg = ci // CG, ci % CG
                            if f < 4:
                                w_ap = wq_g[g][:, cg, ts(f, 128)]
                            else:
                                w_ap = wk_g[g][:, cg, :]
                            x_t = x_a if ci < 8 else x_b
                            nc.tensor.matmul(
                                ps, w_ap, x_t[:, ci % 8, :],
                                start=(ci == 0), stop=(ci == NCC - 1))
                        # RoPE: ACT evacuates PSUM -> bf16 SBUF; partition
                        # shift via bf16 SBUF copies; math in bf16 (DVE
                        # fast modes)
                        straight = rpool.tile([128, TE], BF16, tag="st")
                        nc.scalar.activation(straight, ps, Copy)
                        sh = rpool.tile([128, TE], BF16, tag="sh")
                        for blk in range(4):
                            o = 32 * blk
                            so = o + 32 if blk % 2 == 0 else o - 32
                            nc.vector.tensor_copy(
                                sh[o:o + 32, :], straight[so:so + 32, :])
                        t1 = rpool.tile([128, TE], BF16, tag="t1")
                        nc.vector.tensor_mul(t1, straight, c2_sb[:, ts(e, TE)])
                        t2 = rpool.tile([128, TE], BF16, tag="t2")
                        nc.vector.tensor_mul(t2, sh, s2_sb[:, ts(e, TE)])
                        if f == 4:
                            if padded:
                                for kv in range(2):
                                    nc.vector.tensor_add(
                                        kT_sbp[64 * kv:64 * kv + 64,
                                               kv, ts(e, TE)],
                                        t1[64 * kv:64 * kv + 64, :],
                                        t2[64 * kv:64 * kv + 64, :])
                            else:
                                nc.vector.tensor_add(
                                    kT_sbp[:, ts(e, TE)], t1, t2)
                        else:
                            for s in range(2):
                                h = 2 * f + s
                                kv, slot = h // 4, SLOT[h % 4]
                                nc.vector.tensor_add(
                                    qp_sb[64 * kv:64 * kv + 64, slot,
                                          ts(e, TE)],
                                    t1[64 * s:64 * s + 64, :],
                                    t2[64 * s:64 * s + 64, :])

                    def emit_v_tile(e, jt, x_a, x_b):
                        # v^T directly: x slice stationary, wv moving
                        tt = 4 * e + jt
                        pv = psA.tile([128, TE], F32, tag="acc")
                        for ci in range(NCC):
                            g, cg = ci // CG, ci % CG
                            x_t = x_a if ci < 8 else x_b
                            nc.tensor.matmul(
                                pv[:, 0:KF],
                                x_t[:, ci % 8, ts(jt, 128)],
                                wv_g[g][:, cg, :],
                                start=(ci == 0), stop=(ci == NCC - 1))
                        for kv in range(2):
                            nc.vector.tensor_copy(
                                v1_sb[:, kv, tt, 0:HD], pv[:, ts(kv, HD)])

                    def emit_strip_iter(cstrip, j, po):
                        pss = psS.tile([128, 1024], F32, tag="pss")
                        for kv in range(2):
                            if padded:
                                nc.tensor.matmul(
                                    pss[:, ts(kv, 512)],
                                    kT_sbp[:, kv, ts(j, 128)],
                                    qp_sb[:, :, ts(cstrip, 128)],
                                    start=True, stop=True)
                            else:
                                nc.tensor.matmul(
                                    pss[:, ts(kv, 512)],
                                    kT_sbp[64 * kv:64 * kv + 64, ts(j, 128)],
                                    qp_sb[64 * kv:64 * kv + 64, :,
                                          ts(cstrip, 128)],
                                    start=True, stop=True)
                        pss3 = pss.rearrange("p (a b) -> p a b", a=8)
                        if causal:
                            if j == cstrip:
                                nc.vector.tensor_add(pss3, pss3, maskd_sb)
                        else:
                            mt = mpool.tile([128, 8, 128], F32, tag="mt")
                            for hs in range(8):
                                nc.sync.dma_start(
                                    mt[:, hs, :],
                                    maskT[ts(j, 128), ts(cstrip, 128)])
                            nc.vector.tensor_add(pss3, pss3, mt)
                        p_sb = ppool.tile([128, 1024], BF16, tag="p")
                        nc.scalar.activation(p_sb, pss, Exp, scale=0.125)
                        jmax = cstrip if causal else NTS - 1
                        for kv in range(2):
                            nc.tensor.matmul(
                                po[kv],
                                v1_sb[:, kv, j, :],
                                p_sb[:, ts(kv, 512)],
                                start=(j == 0), stop=(j == jmax))

                    def emit_strip_finish(cstrip, po):
                        att_s = apool.tile([128, 4, 128], BF16, tag="att")
                        for kv in range(2):
                            # sums arrive broadcast in po rows 64:128
                            recb = npool.tile([64, 512], BF16, tag="recb")
                            with nc.allow_low_precision(
                                    reason="softmax denom in bf16 is fine"):
                                nc.vector.reciprocal(recb, po[kv][64:128, :])
                            nc.vector.tensor_mul(
                                att_s[64 * kv:64 * kv + 64, :, :],
                                po[kv][0:64, :].rearrange(
                                    "p (a b) -> p a b", a=4),
                                recb.rearrange("p (a b) -> p a b", a=4))
                        return att_s

                    def emit_o_proj_half(cstrip, att_s, o_full, ecs, last):
                        for ec in ecs:
                            pc = psA.tile([128, TE], F32, tag="acc")
                            for ff in range(4):
                                nc.tensor.matmul(
                                    pc, att_s[:, ff, :],
                                    wo_sb[:, ff, ts(ec, 512)],
                                    start=(ff == 0), stop=(ff == 3))
                            nc.vector.tensor_copy(o_full[:, ts(ec, 512)], pc)
                        if last:
                            nc.sync.dma_start(part[ts(cstrip, 128), :], o_full)

                    # Deferred o_proj is emitted in TWO halves, after the
                    # NEXT strip's 2nd and 5th j-iters: a full 16-matmul
                    # o_proj in the in-order PE queue at a strip boundary
                    # starves ACT of exps (psS double-buffering caps the
                    # exp backlog at ~2 tiles), so each ~1.7us half hides
                    # under the backlog instead.
                    state = {"pending": None, "iters": 0}

                    def flush_pending(upto):
                        # upto: number of halves that may be emitted (2=all)
                        if state["pending"] is None:
                            return
                        c, att_s, o_full, done = state["pending"]
                        while done < upto:
                            ecs = (0, 1) if done == 0 else (2, 3)
                            emit_o_proj_half(c, att_s, o_full, ecs, done == 1)
                            done += 1
                        if done >= 2:
                            state["pending"] = None
                        else:
                            state["pending"] = (c, att_s, o_full, done)

                    def strip_quanta(cstrip):
                        jmax = cstrip if causal else NTS - 1
                        po = {}
                        for kv in range(2):
                            po[kv] = psO.tile([128, 512], F32, tag="po",
                                              name=f"po{kv}")
                        for j in range(jmax + 1):
                            yield ("iter", cstrip, j, po)
                        yield ("finish", cstrip, po)

                    def run_quantum(q):
                        if q[0] == "iter":
                            _, c, j, po = q
                            emit_strip_iter(c, j, po)
                            state["iters"] += 1
                            if state["iters"] == 2:
                                flush_pending(1)
                            elif state["iters"] == 5:
                                flush_pending(2)
                        else:
                            _, c, po = q
                            att_s = emit_strip_finish(c, po)
                            flush_pending(2)
                            o_full = opool.tile([128, C], BF16, tag="ofull")
                            state["pending"] = (c, att_s, o_full, 0)
                            state["iters"] = 0

                    bq = []          # queue of pending B quanta (flattened)

                    def drain(n):
                        for _ in range(n):
                            if bq:
                                run_quantum(bq.pop(0))

                    for e in range(NE):
                        if e == 0:
                            x_a, x_b = x0
                        else:
                            x_a = xpool.tile([128, 8, TE], BF16, tag="xa",
                                             name="x_a")
                            nc.sync.dma_start(x_a, xTd[:, e, 0:8, :])
                            x_b = xpool.tile([128, 8, TE], BF16, tag="xb",
                                             name="x_b")
                            nc.sync.dma_start(x_b, xTd[:, e, 8:16, :])
                        # A/B interleave measured 1.6x SLOWER on HW than
                        # sequential phases (dependency convoys + extra PE
                        # tiling-mode switches), so drain nothing here: all
                        # strips run after the last chunk.
                        per = 0
                        for f in range(5):
                            emit_proj_group(e, f, x_a, x_b)
                            drain(per)
                        for jt in range(4):
                            emit_v_tile(e, jt, x_a, x_b)
                            drain(per)
                        for c in range(4 * e, 4 * e + 4):
                            bq.extend(strip_quanta(c))
                    while bq:
                        run_quantum(bq.pop(0))
                    flush_pending(2)

    nc.finalize()
    return nc


def _bf16(a):
    import concourse.mybir as mybir
    np_bf16 = mybir.dt.np(mybir.dt.bfloat16)
    return np.ascontiguousarray(a.astype(np_bf16))


def _host_inputs(x, freqs_cos, freqs_sin, attention_mask, Wq, Wk, Wv, Wo, causal):
    """Build the 8 per-core input maps (pre-tiled for contiguous DMA)."""
    fc = np.asarray(freqs_cos, np.float32)
    fs = np.asarray(freqs_sin, np.float32)
    mask = np.asarray(attention_mask, np.float32)

    # rope factor tables [128, T]; pattern repeats every 64 partitions
    m_idx = np.tile(np.arange(32), 4)                     # p % 32
    c2v = np.ascontiguousarray(fc.T[m_idx])               # [128, T]
    sgn = np.where((np.arange(128) % 64) < 32, -1.0, 1.0).astype(np.float32)
    s2v = np.ascontiguousarray(fs.T[m_idx] * sgn[:, None])

    # diagonal-block mask, transposed + pre-scaled by 8 (kernel applies *0.125)
    md = (mask[0:128, 0:128].T * 8.0).astype(np.float32)
    maskd = np.ascontiguousarray(
        np.broadcast_to(md[:, None, :], (128, 8, 128))).astype(np.float32)

    qperm = _rope_perm(NH // G)
    kperm = _rope_perm(NKV // G)
    operm = _wo_perm()

    def tile_w(wT, F):
        # [C, F] -> [NG, 128, CG, F], contiguous per partition
        return _bf16(
            np.ascontiguousarray(wT).reshape(NG, CG, 128, F).transpose(0, 2, 1, 3))

    in_maps = []
    for c in range(8):
        b, g = c // 4, c % 4
        xT = np.asarray(x, np.float32)[b].T  # [C, T]
        xtile = _bf16(xT.reshape(NCC, 128, NE, TE).transpose(1, 2, 0, 3))
        wqT = np.asarray(Wq, np.float32)[g * QF:(g + 1) * QF][qperm].T
        wkT = np.asarray(Wk, np.float32)[g * KF:(g + 1) * KF][kperm].T
        wvT = np.asarray(Wv, np.float32)[g * KF:(g + 1) * KF].T
        woT = np.asarray(Wo, np.float32)[:, g * QF:(g + 1) * QF].T[operm]  # [QF, C]
        m = {
            "xTd": xtile,
            "wqd": tile_w(wqT, QF),
            "wkd": tile_w(wkT, KF),
            "wvd": tile_w(wvT, KF),
            "wod": _bf16(woT.reshape(QF // 128, 128, C).transpose(1, 0, 2)),
            "c2": _bf16(c2v),
            "s2": _bf16(s2v),
            "maskd": maskd,
        }
        if not causal:
            m["maskT"] = np.ascontiguousarray(mask.T) * 8.0
        in_maps.append(m)
    return in_maps


def _detect_causal(mask):
    mask = np.asarray(mask)
    neg = mask.min()
    if neg >= -1e7:
        return False
    tril = np.tril(np.ones((T, T), dtype=bool))
    expect = np.where(tril, np.float32(0.0), np.float32(neg))
    return bool(np.array_equal(mask, expect))


def run(inputs, trace=False, qk_mode="padded"):
    from concourse import bass_utils

    causal = _detect_causal(inputs["attention_mask"])
    key = ("prog", causal, qk_mode)
    if key not in _CACHE:
        _CACHE[key] = _build_program(causal, qk_mode=qk_mode)
    nc = _CACHE[key]

    in_maps = _host_inputs(causal=causal, **inputs)
    res = bass_utils.run_bass_kernel_spmd(
        nc, in_maps, core_ids=list(range(8)), trace=trace)

    out = np.empty((B, T, C), np.float32)
    for b in range(B):
        acc = res.results[4 * b]["part"].astype(np.float32)
        for g in range(1, 4):
            acc = acc + res.results[4 * b + g]["part"].astype(np.float32)
        out[b] = acc
    return out, res


def kernel(**inputs):
    out, _ = run(inputs, trace=False)
    return out
